# revision 29
# baseline (speedup 1.0000x reference)
"""Trainium2 Bass kernel for the quantized LM-head (nn_LmHeadTender), v5.

Math (per core, vocab-sharded; V shard = 4000 rows):
    Wl   = dequant_int4(lm_weight)        # per-row scale sw = rowmax/7
    y    = dequant_int4(x, per-(chunk,channel) scale s = tmax*2^(b-13)/7)
    out  = y @ Wl.T
Every scale is factored out of the matmul so both operands are small
integers (times powers of two), exactly representable in fp8 e5m2:
    qw  in [-7, 7]                  (weight ints; |w/s| <= 7 so no clip)
    yq  = qx * 2^(bucket-13)        (activation ints scaled by power of 2)
    out[t, v] = (tmax_c/7) * sw[v] * sum_h yq[t, h] * qw[v, h]
fp8 DoubleRow matmuls (2 k-tiles per pass) run at 2x bf16 (~157 TF/s/core,
measured 216 ns per [128t,512v]xK256 matmul) with exact products
accumulated in f32 PSUM.  The rank-1 output scale m7[chunk]*sw[v] is
applied on the host with the same f32 values the device divided by, so
the device ships raw bf16 logits (32 MB instead of 64).

Per-core pipeline (all phases overlap; PE idle ~2 us/100 us steady state):
  - weights: DMA [v,h] f32 -> DVE rowmax + round(w/sw) -> ACT -magic
    cast fp8 -> PE fp8 transpose (psum element-step 2) -> qw resident in
    SBUF as [h, v] fp8 (125 KB/partition).
  - acts: DMA [t,h] f32 quarters -> DVE max/min/max(-v,u) channel maxes
    -> PE transpose + PSUM reduce into packed [h%128, k] layout ->
    bucket via 13 is_gt against HOST-uploaded thresholds tmax*2^(lv-13)
    (host computes tmax anyway for the output scale) -> PE transposes x
    (f32) -> ACT drains PSUM with fused x*r + magic (per-partition
    scale) -> DVE (q-magic)*2^(b-13) -> y fp8 [h, t].
  - matmul: per chunk 2x8 chains of 16 DoubleRow matmuls vs resident qw
    -> PSUM f32 -> ACT copy to bf16 -> DMA out.
Weight v-blocks interleave with chunk-0/1 chains so the PE starts
matmuls ~13 us in; chunk-2 stats are hoisted before the last weight
group.  3519 us (bf16 baseline) -> 1537 us measured.
"""

import numpy as np

from contextlib import ExitStack

import concourse.bass as bass
import concourse.tile as tile
from concourse import bacc, bass_isa, masks, mybir
from concourse.bass_utils import run_bass_kernel_spmd

FP = mybir.dt.float32
BF = mybir.dt.bfloat16
F8 = mybir.dt.float8e5
I32 = mybir.dt.int32
ALU = mybir.AluOpType
AX = mybir.AxisListType
ACTF = mybir.ActivationFunctionType
DR = mybir.MatmulPerfMode.DoubleRow
RED = bass_isa.ReduceOp

T = 4096            # tokens (2*2048)
H = 4096            # hidden
V = 32000           # vocab
NCORE = 8
VSH = V // NCORE    # 4000 vocab rows per core
CHUNK = 256
NCHUNK = T // CHUNK         # 16
KT = H // 128               # 32 k-tiles
NKP = KT // 2               # 16 k-pairs per chain
HH = H // 2                 # 2048 h-half
DECOMP = 14
QMAX = 7.0
C_MAGIC = 12582912.0        # 1.5 * 2^23: round-to-nearest-even via add/sub
C7 = float(np.float32(1.0) / np.float32(7.0))

WT = 32                     # weight v-tiles (last one has 32 rows)
WT_ROWS = [128] * 31 + [VSH - 31 * 128]
VBS = [512] * 7 + [VSH - 7 * 512]   # v-blocks per chain group (last 416)
NVB = len(VBS)


def _emit(ctx: ExitStack, tc: "tile.TileContext", x_d, w_d, thr_d, out_d):
    nc = tc.nc

    cpool = ctx.enter_context(tc.tile_pool(name="consts", bufs=1))
    ident = cpool.tile([128, 128], FP)
    masks.make_identity(nc, ident[:])
    ident8 = cpool.tile([128, 128], F8)
    nc.vector.tensor_copy(ident8[:], ident[:])
    qw_sb = cpool.tile([128, KT, VSH], F8)   # resident quantized weight [h, v]

    thr_sb = cpool.tile([128, NCHUNK, 14], FP)  # host thresholds + tmax
    nc.sync.dma_start(thr_sb[:], thr_d[:, :].rearrange("p (c l) -> p c l", l=14))

    # persistent pools
    wpool = ctx.enter_context(tc.tile_pool(name="wstg", bufs=2))
    w8pool = ctx.enter_context(tc.tile_pool(name="w8", bufs=2))
    wsml = ctx.enter_context(tc.tile_pool(name="wsml", bufs=2))
    xpool = ctx.enter_context(tc.tile_pool(name="xin", bufs=6))
    m2pool = ctx.enter_context(tc.tile_pool(name="m2", bufs=1))
    spool = ctx.enter_context(tc.tile_pool(name="stats", bufs=1))
    q1pool = ctx.enter_context(tc.tile_pool(name="q1", bufs=2))
    ypool = ctx.enter_context(tc.tile_pool(name="y8", bufs=2))
    opool = ctx.enter_context(tc.tile_pool(name="ostg", bufs=2))

    mmps = ctx.enter_context(tc.tile_pool(name="mmps", bufs=2, space="PSUM"))
    xps = ctx.enter_context(tc.tile_pool(name="xps", bufs=4, space="PSUM"))
    m2ps = ctx.enter_context(tc.tile_pool(name="m2ps", bufs=1, space="PSUM"))
    wps = ctx.enter_context(tc.tile_pool(name="wps", bufs=1, space="PSUM"))

    y8_of = {}
    stats_of = {}

    # ---------------- weight tile m: quantize + transpose into qw_sb ----
    def emit_w_tile(m):
        rows = WT_ROWS[m]
        v0 = m * 128
        wn = []
        for hh in range(2):
            t_ = wpool.tile([128, HH], FP, tag=f"wn{hh}", name=f"wn{hh}", bufs=(2 if hh == 0 else 1))
            nc.sync.dma_start(
                t_[:rows, :], w_d[v0:v0 + rows, hh * HH:(hh + 1) * HH])
            wn.append(t_)
        r1 = wsml.tile([128, 2], FP, tag="r1", name="r1")
        for hh in range(2):
            nc.vector.tensor_reduce(
                r1[:rows, hh:hh + 1], wn[hh][:rows, :], axis=AX.X, op=ALU.max,
                apply_absolute_value=True)
        rmax = wsml.tile([128, 1], FP, tag="rmax", name="rmax")
        nc.vector.tensor_reduce(
            rmax[:rows, :], r1[:rows, :], axis=AX.X, op=ALU.max)
        sw = wsml.tile([128, 1], FP, tag="sw", name="sw")
        nc.vector.tensor_scalar(
            sw[:rows, :], rmax[:rows, :], C7, 1e-9, ALU.mult, ALU.max)
        rw = wsml.tile([128, 1], FP, tag="rw", name="rw")
        nc.vector.reciprocal(rw[:rows, :], sw[:rows, :])
        w8 = []
        for hh in range(2):
            # q + magic  (round-to-nearest-even)
            nc.vector.tensor_scalar(
                wn[hh][:rows, :], wn[hh][:rows, :], rw[:rows, :], C_MAGIC,
                ALU.mult, ALU.add)
            q8 = w8pool.tile([128, HH], F8, tag=f"w8{hh}", name=f"q8{hh}")
            nc.scalar.activation(
                q8[:rows, :], wn[hh][:rows, :], ACTF.Copy, bias=-C_MAGIC,
                scale=1.0)
            w8.append(q8)
        # transpose 32 k-blocks into qw_sb[:, k, v0:v0+rows]
        # (fp8 transpose writes PSUM with element step 2 -> interleaved tile)
        for q in range(4):
            ps = wps.tile([128, 8, 128, 2], F8, tag="wps", name="wps")
            for j in range(8):
                k = q * 8 + j
                src = w8[k // 16][:rows, (k % 16) * 128:(k % 16 + 1) * 128]
                nc.tensor.transpose(
                    ps[:, j, :rows, 0], src, ident8[:rows, :rows])
            nc.scalar.copy(
                qw_sb[:, q * 8:(q + 1) * 8, v0:v0 + rows],
                ps[:, :, :rows, 0])

    # ---------------- activation stats for chunk c ----------------------
    # x arrives as 8 quarter tiles [128, 1024]: (rt, q) pairs. Channel maxes
    # via DVE abs_max of the two row-tiles + PE transpose + PSUM reduce.
    def emit_act_stats_a(c):
        xh = [[None] * 4, [None] * 4]
        cmax = spool.tile([128, KT], FP, tag="cmax", name="cmax")
        QW = H // 4
        for q in range(4):
            for rt in range(2):
                t_ = xpool.tile([128, QW], FP, tag="x", name="x")
                nc.sync.dma_start(
                    t_[:],
                    x_d[c * CHUNK + rt * 128:c * CHUNK + (rt + 1) * 128,
                        q * QW:(q + 1) * QW])
                xh[rt][q] = t_
            for f in range(2):
                sl = slice(f * (QW // 2), (f + 1) * (QW // 2))
                m2q = m2pool.tile([128, QW // 2], FP, tag="m2", name="m2q")
                m2v = m2pool.tile([128, QW // 2], FP, tag="m2v", name="m2v")
                nc.vector.tensor_tensor(
                    m2q[:], xh[0][q][:, sl], xh[1][q][:, sl], op=ALU.max)
                nc.vector.tensor_tensor(
                    m2v[:], xh[0][q][:, sl], xh[1][q][:, sl], op=ALU.min)
                nc.vector.scalar_tensor_tensor(
                    m2q[:], m2v[:], -1.0, m2q[:], op0=ALU.mult, op1=ALU.max)
                ps = m2ps.tile([128, 4, 128], FP, tag="m2ps", name="m2ps")
                for j in range(4):
                    nc.tensor.transpose(
                        ps[:, j, :], m2q[:, j * 128:(j + 1) * 128], ident[:])
                nc.vector.tensor_reduce(
                    cmax[:, q * 8 + f * 4:q * 8 + f * 4 + 4], ps[:],
                    axis=AX.X, op=ALU.max)
        stats_of[c] = [cmax, xh]
        return xh

    def emit_act_stats_b(c):
        cmax = stats_of[c][0]
        thr_c = spool.tile([128, 14], FP, tag="thr_c", name="thr_c")
        nc.vector.tensor_copy(thr_c[:], thr_sb[:, c, :])
        bucket = spool.tile([128, KT], FP, tag="bucket", name="bucket")
        nc.vector.memset(bucket[:], 0.0)
        for lv in range(DECOMP - 1):
            nc.vector.scalar_tensor_tensor(
                bucket[:], cmax[:], thr_c[:, lv:lv + 1], bucket[:],
                op0=ALU.is_gt, op1=ALU.add)
        g = spool.tile([128, KT], FP, tag="g", name="g")
        nc.vector.tensor_scalar(
            g[:], bucket[:], 114.0, 8388608.0, ALU.add, ALU.mult)
        g_i = spool.tile([128, KT], I32, tag="g_i", name="g_i", bufs=2)
        nc.vector.tensor_copy(g_i[:], g[:])
        pw = g_i[:].bitcast(FP)
        ch_thr = spool.tile([128, KT], FP, tag="ch_thr", name="ch_thr")
        nc.vector.tensor_scalar(
            ch_thr[:], pw, thr_c[:, 13:14], None, ALU.mult)
        nc.vector.tensor_scalar(
            ch_thr[:], ch_thr[:], C7, 1e-9, ALU.mult, ALU.max)
        r_t = spool.tile([128, KT], FP, tag="r_t", name="r_t", bufs=2)
        nc.vector.reciprocal(r_t[:], ch_thr[:])
        stats_of[c] = [cmax, stats_of[c][1], g_i, r_t, None]

    # ---------------- transpose + fused quant (per h-quarter) -----------
    def emit_act_transposes(c, qs):
        _, xh, g_i, r_t, bias_k = stats_of[c]
        if qs[0] == 0:
            y8_of[c] = ypool.tile([128, KT, CHUNK], F8, tag="y8", name="y8")
        y8 = y8_of[c]
        for q in qs:
            for kk in range(4):        # k-pairs within quarter
                ps = xps.tile([128, 2, 2, 128], FP, tag="xps", name="ps")
                for ki in range(2):
                    k = q * 8 + kk * 2 + ki
                    for tb in range(2):
                        nc.tensor.transpose(
                            ps[:, ki, tb, :],
                            xh[tb][q][:, (k % 8) * 128:(k % 8 + 1) * 128],
                            ident[:])
                for ki in range(2):
                    k = q * 8 + kk * 2 + ki
                    q1 = q1pool.tile([128, 2, 128], FP, tag="q1", name="q1")
                    nc.scalar.activation(
                        q1[:], ps[:, ki, :, :], ACTF.Copy, bias=C_MAGIC,
                        scale=r_t[:, k:k + 1])
                    dst = y8[:, k, :]
                    q1f = q1[:].rearrange("p a b -> p (a b)")
                    pw_col = g_i[:, k:k + 1].bitcast(FP)
                    nc.vector.tensor_scalar(
                        dst, q1f, C_MAGIC, pw_col,
                        ALU.subtract, ALU.mult)

    # ---------------- matmul chains for chunk c -------------------------
    def emit_chains(c, vbs=None):
        y8 = y8_of[c]
        for tb in range(2):
            for vb in (range(NVB) if vbs is None else vbs):
                wv = VBS[vb]
                ps = mmps.tile([128, 512], FP, tag="mm", name="mmps")
                for kp in range(NKP):
                    nc.tensor.matmul(
                        ps[:, :wv],
                        y8[:, 2 * kp:2 * kp + 2, tb * 128:(tb + 1) * 128],
                        qw_sb[:, 2 * kp:2 * kp + 2, vb * 512:vb * 512 + wv],
                        start=(kp == 0), stop=(kp == NKP - 1), perf_mode=DR)
                stg = opool.tile([128, 512], BF, tag="stg", name="stg")
                nc.scalar.copy(stg[:, :wv], ps[:, :wv])
                nc.gpsimd.dma_start(
                    out_d[c * CHUNK + tb * 128:c * CHUNK + (tb + 1) * 128,
                          vb * 512:vb * 512 + wv],
                    stg[:, :wv])

    # ---------------- emission schedule ---------------------------------
    def emit_stats_and_q0(c):
        emit_act_stats_a(c)
        emit_act_stats_b(c)
        emit_act_transposes(c, [0])

    emit_stats_and_q0(0)
    emit_act_transposes(0, [1, 2, 3])
    emit_stats_and_q0(1)
    emit_act_transposes(1, [1, 2, 3])
    for g in range(NVB):
        for m in range(4 * g, 4 * g + 4):
            emit_w_tile(m)
        emit_chains(0, vbs=[g])
        emit_chains(1, vbs=[g])
    emit_stats_and_q0(2)
    emit_act_transposes(2, [1, 2, 3])
    for c in range(2, NCHUNK):
        if c + 1 < NCHUNK:
            emit_stats_and_q0(c + 1)
        emit_chains(c)
        if c + 1 < NCHUNK:
            emit_act_transposes(c + 1, [1, 2, 3])

_CACHED = None


def _build():
    global _CACHED
    if _CACHED is not None:
        return _CACHED
    nc = bacc.Bacc(
        "TRN2", target_bir_lowering=False, debug=False,
        enable_asserts=False, num_devices=NCORE)
    x_d = nc.dram_tensor("x", (T, H), FP, kind="ExternalInput").ap()
    w_d = nc.dram_tensor("w", (VSH, H), FP, kind="ExternalInput").ap()
    thr_d = nc.dram_tensor("thr", (128, NCHUNK * 14), FP,
                           kind="ExternalInput").ap()
    out_d = nc.dram_tensor("out", (T, VSH), BF, kind="ExternalOutput").ap()
    with tile.TileContext(nc) as tc:
        with ExitStack() as ctx:
            _emit(ctx, tc, x_d, w_d, thr_d, out_d)
    nc.compile()
    _CACHED = nc
    return nc


def kernel(hidden_states: np.ndarray, lm_weight: np.ndarray) -> np.ndarray:
    b, t, h = hidden_states.shape
    assert (b * t, h) == (T, H) and lm_weight.shape == (V, H)
    x_full = np.ascontiguousarray(
        hidden_states.reshape(T, H).astype(np.float32))
    xc = x_full.reshape(NCHUNK, CHUNK * H)
    tmax = np.abs(xc).max(axis=1).astype(np.float32)         # [NCHUNK]
    lv = np.arange(14, dtype=np.float32)
    thr = tmax[:, None] * np.exp2(lv - 13.0)[None, :].astype(np.float32)
    thr[:, 13] = tmax
    thr_np = np.ascontiguousarray(
        np.broadcast_to(thr.reshape(1, NCHUNK * 14).astype(np.float32),
                        (128, NCHUNK * 14)))
    in_maps = []
    for c in range(NCORE):
        shard = np.ascontiguousarray(
            lm_weight[c * VSH:(c + 1) * VSH].astype(np.float32))
        in_maps.append({"x": x_full, "w": shard, "thr": thr_np})
    nc = _build()
    res = run_bass_kernel_spmd(nc, in_maps, core_ids=list(range(NCORE)))

    # host-side rank-1 scale: m7[chunk] * sw[v]
    m7 = (tmax * np.float32(C7)).astype(np.float32)
    m7_col = np.repeat(m7, CHUNK)[:, None]               # [T, 1]
    outs = []
    for c in range(NCORE):
        sw = np.maximum(
            np.abs(in_maps[c]["w"]).max(axis=1) * np.float32(C7),
            np.float32(1e-9)).astype(np.float32)
        arr = res.results[c]["out"].astype(np.float32)
        arr *= m7_col
        arr *= sw[None, :]
        outs.append(arr)
    full = np.concatenate(outs, axis=1)
    return full.reshape(b, t, V)


# revision 30
# speedup vs baseline: 1.0092x; 1.0092x over previous
"""Trainium2 Bass kernel for the quantized LM-head (nn_LmHeadTender), v5.

Math (per core, vocab-sharded; V shard = 4000 rows):
    Wl   = dequant_int4(lm_weight)        # per-row scale sw = rowmax/7
    y    = dequant_int4(x, per-(chunk,channel) scale s = tmax*2^(b-13)/7)
    out  = y @ Wl.T
Every scale is factored out of the matmul so both operands are small
integers (times powers of two), exactly representable in fp8 e5m2:
    qw  in [-7, 7]                  (weight ints; |w/s| <= 7 so no clip)
    yq  = qx * 2^(bucket-13)        (activation ints scaled by power of 2)
    out[t, v] = (tmax_c/7) * sw[v] * sum_h yq[t, h] * qw[v, h]
fp8 DoubleRow matmuls (2 k-tiles per pass) run at 2x bf16 (~157 TF/s/core,
measured 216 ns per [128t,512v]xK256 matmul) with exact products
accumulated in f32 PSUM.  The rank-1 output scale m7[chunk]*sw[v] is
applied on the host with the same f32 values the device divided by, so
the device ships raw bf16 logits (32 MB instead of 64).

Per-core pipeline (all phases overlap; PE idle ~2 us/100 us steady state):
  - weights: DMA [v,h] f32 -> DVE rowmax + round(w/sw) -> ACT -magic
    cast fp8 -> PE fp8 transpose (psum element-step 2) -> qw resident in
    SBUF as [h, v] fp8 (125 KB/partition).
  - acts: DMA [t,h] f32 quarters -> DVE max/min/max(-v,u) channel maxes
    -> PE transpose + PSUM reduce into packed [h%128, k] layout ->
    bucket via 13 is_gt against HOST-uploaded thresholds tmax*2^(lv-13)
    (host computes tmax anyway for the output scale) -> PE transposes x
    (f32) -> ACT drains PSUM with fused x*r + magic (per-partition
    scale) -> DVE (q-magic)*2^(b-13) -> y fp8 [h, t].
  - matmul: per chunk 2x8 chains of 16 DoubleRow matmuls vs resident qw
    -> PSUM f32 -> ACT copy to bf16 -> DMA out.
Weight v-blocks interleave with chunk-0/1 chains so the PE starts
matmuls ~13 us in; chunk-2 stats are hoisted before the last weight
group.  3519 us (bf16 baseline) -> 1537 us measured.
"""

import numpy as np

from contextlib import ExitStack

import concourse.bass as bass
import concourse.tile as tile
from concourse import bacc, bass_isa, masks, mybir
from concourse.bass_utils import run_bass_kernel_spmd

FP = mybir.dt.float32
BF = mybir.dt.bfloat16
F8 = mybir.dt.float8e5
I32 = mybir.dt.int32
ALU = mybir.AluOpType
AX = mybir.AxisListType
ACTF = mybir.ActivationFunctionType
DR = mybir.MatmulPerfMode.DoubleRow
RED = bass_isa.ReduceOp

T = 4096            # tokens (2*2048)
H = 4096            # hidden
V = 32000           # vocab
NCORE = 8
VSH = V // NCORE    # 4000 vocab rows per core
CHUNK = 256
NCHUNK = T // CHUNK         # 16
KT = H // 128               # 32 k-tiles
NKP = KT // 2               # 16 k-pairs per chain
HH = H // 2                 # 2048 h-half
DECOMP = 14
QMAX = 7.0
C_MAGIC = 12582912.0        # 1.5 * 2^23: round-to-nearest-even via add/sub
C7 = float(np.float32(1.0) / np.float32(7.0))

WT = 32                     # weight v-tiles (last one has 32 rows)
WT_ROWS = [128] * 31 + [VSH - 31 * 128]
VBS = [512] * 7 + [VSH - 7 * 512]   # v-blocks per chain group (last 416)
NVB = len(VBS)


def _emit(ctx: ExitStack, tc: "tile.TileContext", x_d, w_d, thr_d, out_d):
    nc = tc.nc

    cpool = ctx.enter_context(tc.tile_pool(name="consts", bufs=1))
    ident = cpool.tile([128, 128], FP)
    masks.make_identity(nc, ident[:])
    ident8 = cpool.tile([128, 128], F8)
    nc.vector.tensor_copy(ident8[:], ident[:])
    qw_sb = cpool.tile([128, KT, VSH], F8)   # resident quantized weight [h, v]

    thr_sb = cpool.tile([128, NCHUNK, 14], FP)  # host thresholds + tmax
    nc.sync.dma_start(thr_sb[:], thr_d[:, :].rearrange("p (c l) -> p c l", l=14))

    # persistent pools
    wpool = ctx.enter_context(tc.tile_pool(name="wstg", bufs=2))
    w8pool = ctx.enter_context(tc.tile_pool(name="w8", bufs=2))
    wsml = ctx.enter_context(tc.tile_pool(name="wsml", bufs=2))
    xpool = ctx.enter_context(tc.tile_pool(name="xin", bufs=6))
    m2pool = ctx.enter_context(tc.tile_pool(name="m2", bufs=1))
    spool = ctx.enter_context(tc.tile_pool(name="stats", bufs=1))
    q1pool = ctx.enter_context(tc.tile_pool(name="q1", bufs=2))
    ypool = ctx.enter_context(tc.tile_pool(name="y8", bufs=2))
    opool = ctx.enter_context(tc.tile_pool(name="ostg", bufs=2))

    mmps = ctx.enter_context(tc.tile_pool(name="mmps", bufs=2, space="PSUM"))
    xps = ctx.enter_context(tc.tile_pool(name="xps", bufs=4, space="PSUM"))
    m2ps = ctx.enter_context(tc.tile_pool(name="m2ps", bufs=1, space="PSUM"))
    wps = ctx.enter_context(tc.tile_pool(name="wps", bufs=1, space="PSUM"))

    y8_of = {}
    stats_of = {}

    # ---------------- weight tile m: quantize + transpose into qw_sb ----
    def emit_w_tile(m):
        rows = WT_ROWS[m]
        v0 = m * 128
        wn = []
        for hh in range(2):
            t_ = wpool.tile([128, HH], FP, tag=f"wn{hh}", name=f"wn{hh}", bufs=(2 if hh == 0 else 1))
            nc.sync.dma_start(
                t_[:rows, :], w_d[v0:v0 + rows, hh * HH:(hh + 1) * HH])
            wn.append(t_)
        r1 = wsml.tile([128, 2], FP, tag="r1", name="r1")
        for hh in range(2):
            nc.vector.tensor_reduce(
                r1[:rows, hh:hh + 1], wn[hh][:rows, :], axis=AX.X, op=ALU.max,
                apply_absolute_value=True)
        rmax = wsml.tile([128, 1], FP, tag="rmax", name="rmax")
        nc.vector.tensor_reduce(
            rmax[:rows, :], r1[:rows, :], axis=AX.X, op=ALU.max)
        sw = wsml.tile([128, 1], FP, tag="sw", name="sw")
        nc.vector.tensor_scalar(
            sw[:rows, :], rmax[:rows, :], C7, 1e-9, ALU.mult, ALU.max)
        rw = wsml.tile([128, 1], FP, tag="rw", name="rw")
        nc.vector.reciprocal(rw[:rows, :], sw[:rows, :])
        w8 = []
        for hh in range(2):
            # q + magic  (round-to-nearest-even)
            nc.vector.tensor_scalar(
                wn[hh][:rows, :], wn[hh][:rows, :], rw[:rows, :], C_MAGIC,
                ALU.mult, ALU.add)
            q8 = w8pool.tile([128, HH], F8, tag=f"w8{hh}", name=f"q8{hh}")
            nc.scalar.activation(
                q8[:rows, :], wn[hh][:rows, :], ACTF.Copy, bias=-C_MAGIC,
                scale=1.0)
            w8.append(q8)
        # transpose 32 k-blocks into qw_sb[:, k, v0:v0+rows]
        # (fp8 transpose writes PSUM with element step 2 -> interleaved tile)
        for q in range(4):
            ps = wps.tile([128, 8, 128, 2], F8, tag="wps", name="wps")
            for j in range(8):
                k = q * 8 + j
                src = w8[k // 16][:rows, (k % 16) * 128:(k % 16 + 1) * 128]
                nc.tensor.transpose(
                    ps[:, j, :rows, 0], src, ident8[:rows, :rows])
            nc.scalar.copy(
                qw_sb[:, q * 8:(q + 1) * 8, v0:v0 + rows],
                ps[:, :, :rows, 0])

    # ---------------- activation stats for chunk c ----------------------
    # x arrives as 8 quarter tiles [128, 1024]: (rt, q) pairs. Channel maxes
    # via DVE abs_max of the two row-tiles + PE transpose + PSUM reduce.
    def emit_act_stats_a(c):
        xh = [[None] * 4, [None] * 4]
        cmax = spool.tile([128, KT], FP, tag="cmax", name="cmax")
        QW = H // 4
        for q in range(4):
            for rt in range(2):
                t_ = xpool.tile([128, QW], FP, tag="x", name="x")
                nc.sync.dma_start(
                    t_[:],
                    x_d[c * CHUNK + rt * 128:c * CHUNK + (rt + 1) * 128,
                        q * QW:(q + 1) * QW])
                xh[rt][q] = t_
            for f in range(2):
                sl = slice(f * (QW // 2), (f + 1) * (QW // 2))
                m2q = m2pool.tile([128, QW // 2], FP, tag="m2", name="m2q")
                m2v = m2pool.tile([128, QW // 2], FP, tag="m2v", name="m2v")
                nc.vector.tensor_tensor(
                    m2q[:], xh[0][q][:, sl], xh[1][q][:, sl], op=ALU.max)
                nc.vector.tensor_tensor(
                    m2v[:], xh[0][q][:, sl], xh[1][q][:, sl], op=ALU.min)
                nc.vector.scalar_tensor_tensor(
                    m2q[:], m2v[:], -1.0, m2q[:], op0=ALU.mult, op1=ALU.max)
                ps = m2ps.tile([128, 4, 128], FP, tag="m2ps", name="m2ps")
                for j in range(4):
                    nc.tensor.transpose(
                        ps[:, j, :], m2q[:, j * 128:(j + 1) * 128], ident[:])
                nc.vector.tensor_reduce(
                    cmax[:, q * 8 + f * 4:q * 8 + f * 4 + 4], ps[:],
                    axis=AX.X, op=ALU.max)
        stats_of[c] = [cmax, xh]
        return xh

    def emit_act_stats_b(c):
        cmax = stats_of[c][0]
        thr_c = spool.tile([128, 14], FP, tag="thr_c", name="thr_c")
        nc.vector.tensor_copy(thr_c[:], thr_sb[:, c, :])
        bucket = spool.tile([128, KT], FP, tag="bucket", name="bucket")
        nc.vector.memset(bucket[:], 0.0)
        for lv in range(DECOMP - 1):
            nc.vector.scalar_tensor_tensor(
                bucket[:], cmax[:], thr_c[:, lv:lv + 1], bucket[:],
                op0=ALU.is_gt, op1=ALU.add)
        g = spool.tile([128, KT], FP, tag="g", name="g")
        nc.vector.tensor_scalar(
            g[:], bucket[:], 114.0, 8388608.0, ALU.add, ALU.mult)
        g_i = spool.tile([128, KT], I32, tag="g_i", name="g_i", bufs=2)
        nc.vector.tensor_copy(g_i[:], g[:])
        pw = g_i[:].bitcast(FP)
        ch_thr = spool.tile([128, KT], FP, tag="ch_thr", name="ch_thr")
        nc.vector.tensor_scalar(
            ch_thr[:], pw, thr_c[:, 13:14], None, ALU.mult)
        nc.vector.tensor_scalar(
            ch_thr[:], ch_thr[:], C7, 1e-9, ALU.mult, ALU.max)
        r_t = spool.tile([128, KT], FP, tag="r_t", name="r_t", bufs=2)
        nc.vector.reciprocal(r_t[:], ch_thr[:])
        stats_of[c] = [cmax, stats_of[c][1], g_i, r_t, None]

    # ---------------- transpose + fused quant (per h-quarter) -----------
    def emit_act_transposes(c, qs):
        _, xh, g_i, r_t, bias_k = stats_of[c]
        if qs[0] == 0:
            y8_of[c] = ypool.tile([128, KT, CHUNK], F8, tag="y8", name="y8")
        y8 = y8_of[c]
        for q in qs:
            for kk in range(4):        # k-pairs within quarter
                ps = xps.tile([128, 2, 2, 128], FP, tag="xps", name="ps")
                for ki in range(2):
                    k = q * 8 + kk * 2 + ki
                    for tb in range(2):
                        nc.tensor.transpose(
                            ps[:, ki, tb, :],
                            xh[tb][q][:, (k % 8) * 128:(k % 8 + 1) * 128],
                            ident[:])
                for ki in range(2):
                    k = q * 8 + kk * 2 + ki
                    q1 = q1pool.tile([128, 2, 128], FP, tag="q1", name="q1")
                    nc.scalar.activation(
                        q1[:], ps[:, ki, :, :], ACTF.Copy, bias=C_MAGIC,
                        scale=r_t[:, k:k + 1])
                    dst = y8[:, k, :]
                    q1f = q1[:].rearrange("p a b -> p (a b)")
                    pw_col = g_i[:, k:k + 1].bitcast(FP)
                    nc.vector.tensor_scalar(
                        dst, q1f, C_MAGIC, pw_col,
                        ALU.subtract, ALU.mult)

    # ---------------- matmul chains for chunk c -------------------------
    def emit_chains(c, vbs=None):
        y8 = y8_of[c]
        for tb in range(2):
            for vb in (range(NVB) if vbs is None else vbs):
                wv = VBS[vb]
                ps = mmps.tile([128, 512], FP, tag="mm", name="mmps")
                for kp in range(NKP):
                    nc.tensor.matmul(
                        ps[:, :wv],
                        y8[:, 2 * kp:2 * kp + 2, tb * 128:(tb + 1) * 128],
                        qw_sb[:, 2 * kp:2 * kp + 2, vb * 512:vb * 512 + wv],
                        start=(kp == 0), stop=(kp == NKP - 1), perf_mode=DR)
                stg = opool.tile([128, 512], BF, tag="stg", name="stg")
                nc.scalar.copy(stg[:, :wv], ps[:, :wv])
                nc.sync.dma_start(
                    out_d[c * CHUNK + tb * 128:c * CHUNK + (tb + 1) * 128,
                          vb * 512:vb * 512 + wv],
                    stg[:, :wv])

    # ---------------- emission schedule ---------------------------------
    def emit_stats_and_q0(c):
        emit_act_stats_a(c)
        emit_act_stats_b(c)
        emit_act_transposes(c, [0])

    emit_stats_and_q0(0)
    emit_act_transposes(0, [1, 2, 3])
    emit_stats_and_q0(1)
    emit_act_transposes(1, [1, 2, 3])
    for g in range(NVB):
        for m in range(4 * g, 4 * g + 4):
            emit_w_tile(m)
        emit_chains(0, vbs=[g])
        emit_chains(1, vbs=[g])
    emit_stats_and_q0(2)
    emit_act_transposes(2, [1, 2, 3])
    for c in range(2, NCHUNK):
        if c + 1 < NCHUNK:
            emit_stats_and_q0(c + 1)
        emit_chains(c)
        if c + 1 < NCHUNK:
            emit_act_transposes(c + 1, [1, 2, 3])

_CACHED = None


def _build():
    global _CACHED
    if _CACHED is not None:
        return _CACHED
    nc = bacc.Bacc(
        "TRN2", target_bir_lowering=False, debug=False,
        enable_asserts=False, num_devices=NCORE)
    x_d = nc.dram_tensor("x", (T, H), FP, kind="ExternalInput").ap()
    w_d = nc.dram_tensor("w", (VSH, H), FP, kind="ExternalInput").ap()
    thr_d = nc.dram_tensor("thr", (128, NCHUNK * 14), FP,
                           kind="ExternalInput").ap()
    out_d = nc.dram_tensor("out", (T, VSH), BF, kind="ExternalOutput").ap()
    with tile.TileContext(nc) as tc:
        with ExitStack() as ctx:
            _emit(ctx, tc, x_d, w_d, thr_d, out_d)
    nc.compile()
    _CACHED = nc
    return nc


def kernel(hidden_states: np.ndarray, lm_weight: np.ndarray) -> np.ndarray:
    b, t, h = hidden_states.shape
    assert (b * t, h) == (T, H) and lm_weight.shape == (V, H)
    x_full = np.ascontiguousarray(
        hidden_states.reshape(T, H).astype(np.float32))
    xc = x_full.reshape(NCHUNK, CHUNK * H)
    tmax = np.abs(xc).max(axis=1).astype(np.float32)         # [NCHUNK]
    lv = np.arange(14, dtype=np.float32)
    thr = tmax[:, None] * np.exp2(lv - 13.0)[None, :].astype(np.float32)
    thr[:, 13] = tmax
    thr_np = np.ascontiguousarray(
        np.broadcast_to(thr.reshape(1, NCHUNK * 14).astype(np.float32),
                        (128, NCHUNK * 14)))
    in_maps = []
    for c in range(NCORE):
        shard = np.ascontiguousarray(
            lm_weight[c * VSH:(c + 1) * VSH].astype(np.float32))
        in_maps.append({"x": x_full, "w": shard, "thr": thr_np})
    nc = _build()
    res = run_bass_kernel_spmd(nc, in_maps, core_ids=list(range(NCORE)))

    # host-side rank-1 scale: m7[chunk] * sw[v]
    m7 = (tmax * np.float32(C7)).astype(np.float32)
    m7_col = np.repeat(m7, CHUNK)[:, None]               # [T, 1]
    outs = []
    for c in range(NCORE):
        sw = np.maximum(
            np.abs(in_maps[c]["w"]).max(axis=1) * np.float32(C7),
            np.float32(1e-9)).astype(np.float32)
        arr = res.results[c]["out"].astype(np.float32)
        arr *= m7_col
        arr *= sw[None, :]
        outs.append(arr)
    full = np.concatenate(outs, axis=1)
    return full.reshape(b, t, V)
